# revision 1
# baseline (speedup 1.0000x reference)
"""Trainium2 Bass kernel for nn_ExpandEvecs.

Reference computation (fp32):
    evecs [B=4, C=1, N=1024, K=16]
    outers[b,k,c,n,m] = evecs[b,c,n,k] * evecs[b,c,m,k]
    cube = cumsum(outers, axis=k)          -> [B, K, C, N, N]
    out  = cube.reshape(B, K*C, N, N)      -> [4, 16, 1024, 1024]

i.e. out[b, k] = X[:, :k+1] @ X[:, :k+1]^T with X = evecs[b, 0]  [N, K].

Sharding: 8 cores, core c -> (b = c//2, level-half = c%2). Each core
computes 8 output slabs [1024, 1024] = 32 MB and writes them out; the
per-core level subset is encoded in the DATA (zero-masked fp16 rhs
tensors prepared on host), so the SPMD program is identical on all
cores. See _build_bass_hybrid for the kernel structure. Measured
~110 us HW exec per core (DMA-write roofline ~93 us at ~358 GB/s),
scaled absmax error ~2.3e-7 vs the fp32 reference.
"""

import sys

if "/opt/trn_rl_repo" not in sys.path:
    sys.path.insert(0, "/opt/trn_rl_repo")

import numpy as np

B = 4          # batch
NLEV = 16      # total levels (K)
N = 1024       # vector length
KC = 16        # contract dim (= K)
NCORES = 8
LEV = 8        # levels per core
P = 128        # partition tile (row chunk)
RC = N // P    # 8 row chunks
FH = 512       # psum free dim (col half)
NH = N // FH   # 2 col halves

_nc_cache = {}


def build_bass(mm_dtype="hybrid"):
    if mm_dtype == "hybrid":
        return _build_bass_hybrid()
    if mm_dtype == "hybrid_sim":
        return _build_bass_hybrid(sim_safe=True)
    if mm_dtype == "float16x3":
        return _build_bass_f16x3()
    return _build_bass_fp32(mm_dtype)


def _build_bass_hybrid(nchain=5, sim_safe=False):
    """Hybrid PE + vector-engine kernel, DMA-write-bound target.

    Work unit = one full output row block [128, 1024] (level j, row
    chunk i) = 512 KB contiguous in DRAM (4 KB per-partition DMA
    descriptors). The 8 row chunks per core split into:
      - PE chunks (i >= nchain): each level is two [128,512] 3-pass
        fp16 hi/lo matmuls (X(x)X ~= hh+hl+lh exactly in fp32 PSUM),
        copied PSUM->SBUF by the Scalar engine.
      - chain chunks (i < nchain): cumsum trick -- level j = level j-1
        + y_j (x) x_j in exact fp32: full-width per-partition-scalar
        multiply + add, both on the Vector engine, into a fresh tile
        each level (so outgoing DMAs never block the chain). Chains are
        seeded by the ordinary level-0 PE block (the level-0 mask
        already covers the levels below this core's range, so the SPMD
        program stays uniform across cores).
    The y_j rows are broadcast across partitions on-chip: 4 DMAs seed
    partitions 0/32/64/96, then a per-level DVE stream_shuffle with an
    all-zeros mask replicates within each 32-partition quadrant.
    Measured engine busy per core: PE ~74us, DVE ~81us, ACT ~64us,
    Sync (DMA issue) ~98us, under the ~32 MB DMA-write roofline.
    """
    import concourse.mybir as mybir
    import concourse.tile as tile
    from concourse import bacc

    dt = mybir.dt
    nc = bacc.Bacc(None, target_bir_lowering=False)
    xrh = nc.dram_tensor("xrh", [KC, N], dt.float16, kind="ExternalInput")
    xrl = nc.dram_tensor("xrl", [KC, N], dt.float16, kind="ExternalInput")
    xmh = nc.dram_tensor("xmh", [KC, LEV * N], dt.float16, kind="ExternalInput")
    xml = nc.dram_tensor("xml", [KC, LEV * N], dt.float16, kind="ExternalInput")
    yb32 = nc.dram_tensor("yb32", [1, LEV * N], dt.float32, kind="ExternalInput")
    xc32 = nc.dram_tensor("xc32", [P, RC * LEV], dt.float32, kind="ExternalInput")
    out = nc.dram_tensor("out", [LEV, N, N], dt.float32, kind="ExternalOutput")

    chain_chunks = list(range(nchain))
    pe_chunks = list(range(nchain, RC))

    with tile.TileContext(nc) as tc:
        with (
            tc.tile_pool(name="xin", bufs=1) as xin,
            tc.tile_pool(name="ybb", bufs=1) as ybbp,
            tc.tile_pool(name="stage", bufs=6) as stage_pool,
            tc.tile_pool(name="chstg", bufs=3) as chp,
            tc.tile_pool(name="tmp", bufs=10) as tmpp,
            tc.tile_pool(name="psA", bufs=8, space="PSUM") as psA,
        ):
            def load(dram, shape, dtype, tag):
                t = xin.tile(shape, dtype, tag=tag)
                nc.sync.dma_start(t[:], dram[:])
                return t

            xrh_t = load(xrh, [KC, N], dt.float16, "xrh")
            xrl_t = load(xrl, [KC, N], dt.float16, "xrl")
            # first two levels' masks early so the PE can start ASAP
            hm_early, lm_early = [], []
            for j in range(2):
                th = xin.tile([KC, N], dt.float16, tag=f"xmh{j}")
                nc.sync.dma_start(th[:], xmh[:, j * N:(j + 1) * N])
                hm_early.append(th)
                tl = xin.tile([KC, N], dt.float16, tag=f"xml{j}")
                nc.sync.dma_start(tl[:], xml[:, j * N:(j + 1) * N])
                lm_early.append(tl)
            xc32_t = load(xc32, [P, RC * LEV], dt.float32, "xc32")

            # Seed the fp32 level rows into partitions 0/32/64/96, then a
            # per-level DVE stream_shuffle (mask all-zeros) broadcasts them
            # across each 32-partition quadrant -- no HBM re-reads.
            ybq = ybbp.tile([P, LEV * N], dt.float32, tag="ybq")
            if sim_safe:
                # CoreSim flags reads of never-written partitions; HW
                # shuffle only uses mask-selected lanes, so skip there.
                nc.gpsimd.memset(ybq[:], 0.0)
            for q in range(4):
                nc.sync.dma_start(ybq[q * 32:q * 32 + 1, :], yb32[:])
            ybbj = {}
            for j in range(1, LEV):
                t = ybbp.tile([P, N], dt.float32, tag=f"ybb{j % 2}")
                nc.vector.stream_shuffle(
                    t[:], ybq[:, j * N:(j + 1) * N], [0] * 32)
                ybbj[j] = t
            hm, lm = list(hm_early), list(lm_early)
            for j in range(2, LEV):
                th = xin.tile([KC, N], dt.float16, tag=f"xmh{j}")
                nc.sync.dma_start(th[:], xmh[:, j * N:(j + 1) * N])
                hm.append(th)
                tl = xin.tile([KC, N], dt.float16, tag=f"xml{j}")
                nc.sync.dma_start(tl[:], xml[:, j * N:(j + 1) * N])
                lm.append(tl)

            def mm3(ps, si, rh, rl, sh):
                nc.tensor.matmul(ps[:], xrh_t[:, si], rh[:, sh],
                                 start=True, stop=False)
                nc.tensor.matmul(ps[:], xrh_t[:, si], rl[:, sh],
                                 start=False, stop=False)
                nc.tensor.matmul(ps[:], xrl_t[:, si], rh[:, sh],
                                 start=False, stop=True)

            def pe_block(i, j, pool, tag):
                si = slice(i * P, (i + 1) * P)
                stg = pool.tile([P, N], dt.float32, tag=tag)
                for h in range(NH):
                    sh = slice(h * FH, (h + 1) * FH)
                    ps = psA.tile([P, FH], dt.float32, tag="pss")
                    mm3(ps, si, hm[j], lm[j], sh)
                    nc.scalar.copy(stg[:, sh], ps[:])
                nc.sync.dma_start(out[j, i * P:(i + 1) * P, :], stg[:])
                return stg

            # level 0: every chunk is a PE block; chain chunks keep the
            # tile as their chain seed (level-0 mask covers the levels
            # below this core's range, so it doubles as the base)
            prev = {}
            for i in chain_chunks:
                prev[i] = pe_block(i, 0, chp, f"cs{i}")
            for i in pe_chunks:
                pe_block(i, 0, stage_pool, "stg")

            for j in range(1, LEV):
                tmps = {}
                for i in chain_chunks:
                    tmp = tmpp.tile([P, N], dt.float32, tag="tmp")
                    scl = xc32_t[:, i * LEV + j: i * LEV + j + 1]
                    nc.vector.tensor_scalar_mul(tmp[:], ybbj[j][:], scl)
                    tmps[i] = tmp
                for i in chain_chunks:
                    cur = chp.tile([P, N], dt.float32, tag=f"cs{i}")
                    nc.vector.tensor_add(cur[:], prev[i][:], tmps[i][:])
                    prev[i] = cur
                    nc.sync.dma_start(out[j, i * P:(i + 1) * P, :], cur[:])
                for i in pe_chunks:
                    pe_block(i, j, stage_pool, "stg")
    nc.compile()
    return nc


def _build_bass_f16x3():
    """fp16 hi/lo split: X (x) X ~= hi(x)hi + hi(x)lo + lo(x)hi, each a
    1-cycle/row fp16 matmul accumulating in fp32 PSUM. ~1e-6 rel err."""
    import concourse.mybir as mybir
    import concourse.tile as tile
    from concourse import bacc

    dt = mybir.dt
    nc = bacc.Bacc(None, target_bir_lowering=False)
    xrh = nc.dram_tensor("xrh", [KC, N], dt.float16, kind="ExternalInput")
    xrl = nc.dram_tensor("xrl", [KC, N], dt.float16, kind="ExternalInput")
    xmh = nc.dram_tensor("xmh", [KC, LEV * N], dt.float16, kind="ExternalInput")
    xml = nc.dram_tensor("xml", [KC, LEV * N], dt.float16, kind="ExternalInput")
    out = nc.dram_tensor("out", [LEV, N, N], dt.float32, kind="ExternalOutput")

    with tile.TileContext(nc) as tc:
        with (
            tc.tile_pool(name="xin", bufs=1) as xin,
            tc.tile_pool(name="stage", bufs=6) as stage_pool,
            tc.tile_pool(name="psum", bufs=4, space="PSUM") as psum_pool,
        ):
            xrh_t = xin.tile([KC, N], dt.float16, tag="xrh")
            nc.sync.dma_start(xrh_t[:], xrh[:])
            xrl_t = xin.tile([KC, N], dt.float16, tag="xrl")
            nc.sync.dma_start(xrl_t[:], xrl[:])
            hm, lm = list(hm_early), list(lm_early)
            for j in range(2, LEV):
                th = xin.tile([KC, N], dt.float16, tag=f"xmh{j}")
                nc.sync.dma_start(th[:], xmh[:, j * N:(j + 1) * N])
                hm.append(th)
                tl = xin.tile([KC, N], dt.float16, tag=f"xml{j}")
                nc.sync.dma_start(tl[:], xml[:, j * N:(j + 1) * N])
                lm.append(tl)

            for i in range(RC):
                si = slice(i * P, (i + 1) * P)
                for j in range(LEV):
                    e = j % 2
                    stg = stage_pool.tile([P, N], dt.float32, tag=f"stg{e}")
                    for h in range(NH):
                        sh = slice(h * FH, (h + 1) * FH)
                        ps = psum_pool.tile([P, FH], dt.float32,
                                            tag="psv" if e == 0 else "pss")
                        nc.tensor.matmul(ps[:], xrh_t[:, si], hm[j][:, sh],
                                         start=True, stop=False)
                        nc.tensor.matmul(ps[:], xrh_t[:, si], lm[j][:, sh],
                                         start=False, stop=False)
                        nc.tensor.matmul(ps[:], xrl_t[:, si], hm[j][:, sh],
                                         start=False, stop=True)
                        if e == 0:
                            nc.vector.tensor_copy(stg[:, sh], ps[:])
                        else:
                            nc.scalar.copy(stg[:, sh], ps[:])
                    nc.sync.dma_start(out[j, i * P:(i + 1) * P, :], stg[:])
    nc.compile()
    return nc


def _build_bass_fp32(mm_dtype):
    import concourse.mybir as mybir
    import concourse.tile as tile
    from concourse import bacc

    dt = mybir.dt
    nc = bacc.Bacc(None, target_bir_lowering=False)
    xr = nc.dram_tensor("xr", [KC, N], dt.float32, kind="ExternalInput")
    xm = nc.dram_tensor("xm", [KC, LEV * N], dt.float32, kind="ExternalInput")
    out = nc.dram_tensor("out", [LEV, N, N], dt.float32, kind="ExternalOutput")
    mmdt = getattr(dt, mm_dtype)

    with tile.TileContext(nc) as tc:
        with (
            tc.tile_pool(name="xin", bufs=1) as xin,
            tc.tile_pool(name="stage", bufs=6) as stage_pool,
            tc.tile_pool(name="psum", bufs=4, space="PSUM") as psum_pool,
        ):
            # Level j is handled end-to-end by one copy engine
            # (j even -> Vector, j odd -> Scalar) so that every matmul /
            # DMA instruction needs at most ONE semaphore wait (trn2
            # matmul + DMA instructions have a single wait slot).
            def conv_copy(engine, dst, src):
                if engine == 0:
                    nc.vector.tensor_copy(dst, src)
                else:
                    nc.scalar.copy(dst, src)

            xr_raw = xin.tile([KC, N], dt.float32, tag="xr_raw")
            nc.sync.dma_start(xr_raw[:], xr[:])
            if mmdt == dt.float32:
                xr_ts = [xr_raw, xr_raw]
            else:
                # fp32r operands must be rounded by a producing compute
                # op; one rounded copy per engine parity.
                xr_ts = []
                for e in range(2):
                    t = xin.tile([KC, N], mmdt, tag=f"xr{e}")
                    conv_copy(e, t[:], xr_raw[:])
                    xr_ts.append(t)
            xm_ts = []
            for j in range(LEV):
                raw = xin.tile([KC, N], dt.float32, tag=f"xm{j}_raw")
                nc.sync.dma_start(raw[:], xm[:, j * N:(j + 1) * N])
                if mmdt == dt.float32:
                    xm_ts.append(raw)
                else:
                    t = xin.tile([KC, N], mmdt, tag=f"xm{j}")
                    conv_copy(j % 2, t[:], raw[:])
                    xm_ts.append(t)

            for i in range(RC):
                for j in range(LEV):
                    e = j % 2
                    stg = stage_pool.tile([P, N], dt.float32, tag=f"stg{e}")
                    for h in range(NH):
                        # Dedicated PSUM banks per copy engine so each
                        # matmul's slot-release wait involves only one
                        # engine's semaphore.
                        ps = psum_pool.tile([P, FH], dt.float32,
                                            tag="psv" if e == 0 else "pss")
                        nc.tensor.matmul(
                            ps[:],
                            xr_ts[e][:, i * P:(i + 1) * P],
                            xm_ts[j][:, h * FH:(h + 1) * FH],
                            start=True,
                            stop=True,
                        )
                        conv_copy(e, stg[:, h * FH:(h + 1) * FH], ps[:])
                    nc.sync.dma_start(out[j, i * P:(i + 1) * P, :], stg[:])
    nc.compile()
    return nc


def _get_nc(mm_dtype):
    if mm_dtype not in _nc_cache:
        _nc_cache[mm_dtype] = build_bass(mm_dtype)
    return _nc_cache[mm_dtype]


def _split16(a):
    """fp32 -> (hi, lo) float16 with a ~= hi + lo."""
    hi = a.astype(np.float16)
    lo = (a - hi.astype(np.float32)).astype(np.float16)
    return hi, lo


def host_inputs(evecs, mm_dtype="hybrid"):
    """Per-core input maps. Core c -> (b=c//2, half=c%2)."""
    in_maps = []
    for c in range(NCORES):
        b, half = divmod(c, 2)
        X = evecs[b, 0].astype(np.float32)                 # [1024, 16]
        xT = np.ascontiguousarray(X.T)                     # [16, 1024]
        xmask = np.zeros((KC, LEV, N), np.float32)
        for j in range(LEV):
            kmax = half * LEV + j  # global level index
            xmask[: kmax + 1, j, :] = xT[: kmax + 1]
        xmask = xmask.reshape(KC, LEV * N)
        if mm_dtype == "hybrid":
            xrh, xrl = _split16(xT)
            xmh, xml = _split16(xmask)
            yb32 = np.ascontiguousarray(
                xT[half * LEV: half * LEV + LEV].reshape(1, LEV * N))
            # per-partition scalars: xc32[p, i*LEV+j] = X[i*128+p, half*LEV+j]
            xc32 = np.ascontiguousarray(
                X.reshape(RC, P, KC)[:, :, half * LEV: half * LEV + LEV]
                .transpose(1, 0, 2).reshape(P, RC * LEV))
            in_maps.append({
                "xrh": np.ascontiguousarray(xrh),
                "xrl": np.ascontiguousarray(xrl),
                "xmh": np.ascontiguousarray(xmh),
                "xml": np.ascontiguousarray(xml),
                "yb32": yb32,
                "xc32": xc32,
            })
        elif mm_dtype == "float16x3":
            xrh, xrl = _split16(xT)
            xmh, xml = _split16(xmask)
            in_maps.append({
                "xrh": np.ascontiguousarray(xrh),
                "xrl": np.ascontiguousarray(xrl),
                "xmh": np.ascontiguousarray(xmh),
                "xml": np.ascontiguousarray(xml),
            })
        else:
            in_maps.append({"xr": xT, "xm": np.ascontiguousarray(xmask)})
    return in_maps


def run(evecs, trace=False, mm_dtype="hybrid", **spmd_kwargs):
    from concourse.bass_utils import run_bass_kernel_spmd

    nc = _get_nc(mm_dtype)
    in_maps = host_inputs(evecs, mm_dtype)
    r = run_bass_kernel_spmd(
        nc, in_maps, core_ids=list(range(NCORES)), trace=trace, **spmd_kwargs
    )
    full = np.empty((B, NLEV, N, N), np.float32)
    for c in range(NCORES):
        b, half = divmod(c, 2)
        full[b, half * LEV:(half + 1) * LEV] = r.results[c]["out"]
    return full, r


def kernel(**inputs):
    evecs = np.asarray(inputs["evecs"])
    full, _ = run(evecs)
    return full



# revision 2
# speedup vs baseline: 1.9468x; 1.9468x over previous
"""Trainium2 Bass kernel for nn_ExpandEvecs.

Reference computation (fp32):
    evecs [B=4, C=1, N=1024, K=16]
    outers[b,k,c,n,m] = evecs[b,c,n,k] * evecs[b,c,m,k]
    cube = cumsum(outers, axis=k)          -> [B, K, C, N, N]
    out  = cube.reshape(B, K*C, N, N)      -> [4, 16, 1024, 1024]

i.e. out[b, k] = X[:, :k+1] @ X[:, :k+1]^T with X = evecs[b, 0]  [N, K].

Key optimizations vs a full fp32 writeout (tolerance is rel_err < 2e-2,
we land ~1e-4):
  1. fp16 output: halves HBM write traffic. Host upcasts to fp32.
  2. Symmetry: out[b,k] is symmetric, so only the upper block-triangle
     (36 of 64 [128,128] blocks per slab, 56.25%) is computed + written.
     Host mirrors the strictly-lower blocks via transpose.
  3. Single-pass fp16 matmul (no hi/lo split): X_h (x) X_h in fp32 PSUM
     gives ~1e-3 elementwise error, far inside the gate.

Sharding: 8 cores, core c -> (b = c//2, level-half = c%2); 8 levels per
core. The level subset is encoded in the DATA (per-level masked fp16
moving operand xm), so the SPMD program is identical on all cores.

Per core per level: 12 matmuls cover the 8 row-chunks' upper trapezoids
(4608 cols total), PSUM->SBUF copies are split between the Vector (2048
cols) and Scalar (2560 cols) engines into two per-engine packed stage
tiles, each drained by ONE output DMA (so each DMA waits on exactly one
engine semaphore). Per-core writes: 8 levels x 9 KiB/partition = 9 MiB
-> ~26 us at the ~360 GB/s DMA roofline (vs 32 MiB / ~93 us for the
fp32 full-slab baseline).
"""

import sys

if "/opt/trn_rl_repo" not in sys.path:
    sys.path.insert(0, "/opt/trn_rl_repo")

import numpy as np

B = 4          # batch
NLEV = 16      # total levels (K)
N = 1024       # vector length
KC = 16        # contract dim (= K)
NCORES = 8
LEV = 8        # levels per core
P = 128        # partition tile (row chunk)
RC = N // P    # 8 row chunks
FH = 512       # psum free dim (max matmul cols per bank)

# Upper-trapezoid pieces per level: (row chunk i, col start, col end) in
# the [1024, 1024] slab; chunk i needs cols [i*128, 1024). Split into a
# Vector-engine set (2048 cols) and a Scalar-engine set (2560 cols),
# balancing DVE @0.96GHz vs ACT @1.2GHz copy throughput.
VPIECES = [(0, 0, 512), (0, 512, 1024), (1, 128, 640),
           (3, 896, 1024), (6, 768, 1024), (7, 896, 1024)]
APIECES = [(1, 640, 1024), (2, 256, 768), (2, 768, 1024),
           (3, 384, 896), (4, 512, 1024), (5, 640, 1024)]
VW = sum(ce - cs for _, cs, ce in VPIECES)   # 2048
AW = sum(ce - cs for _, cs, ce in APIECES)   # 2560

_nc_cache = {}


def build_bass(variant="tri16"):
    import concourse.mybir as mybir
    import concourse.tile as tile
    from concourse import bacc

    dt = mybir.dt
    nc = bacc.Bacc(None, target_bir_lowering=False)
    xr = nc.dram_tensor("xr", [KC, N], dt.float16, kind="ExternalInput")
    xm = nc.dram_tensor("xm", [KC, LEV * N], dt.float16, kind="ExternalInput")
    outv = nc.dram_tensor("outv", [LEV, P, VW], dt.float16,
                          kind="ExternalOutput")
    outa = nc.dram_tensor("outa", [LEV, P, AW], dt.float16,
                          kind="ExternalOutput")

    # Interleave V/A pieces so both copy engines get work early and the
    # PE alternates between the two stage tiles.
    order = []
    for k in range(max(len(VPIECES), len(APIECES))):
        if k < len(VPIECES):
            order.append(("v", VPIECES[k]))
        if k < len(APIECES):
            order.append(("a", APIECES[k]))

    with tile.TileContext(nc) as tc:
        with (
            tc.tile_pool(name="xin", bufs=1) as xin,
            tc.tile_pool(name="stage", bufs=1) as stg,
            tc.tile_pool(name="ps", bufs=8, space="PSUM") as psp,
        ):
            xr_t = xin.tile([KC, N], dt.float16, tag="xr")
            nc.sync.dma_start(xr_t[:], xr[:])
            xm_t = xin.tile([KC, LEV * N], dt.float16, tag="xm")
            # level-0 slice first so the PE can start ASAP
            nc.sync.dma_start(xm_t[:, 0:N], xm[:, 0:N])
            nc.sync.dma_start(xm_t[:, N:], xm[:, N:])

            for j in range(LEV):
                sv = stg.tile([P, VW], dt.float16, tag=f"sv{j}")
                sa = stg.tile([P, AW], dt.float16, tag=f"sa{j}")
                offs = {"v": 0, "a": 0}
                tiles = {"v": sv, "a": sa}
                for eng, (i, cs, ce) in order:
                    w = ce - cs
                    ps = psp.tile([P, FH], dt.float32, tag="ps")
                    nc.tensor.matmul(
                        ps[:, :w],
                        xr_t[:, i * P:(i + 1) * P],
                        xm_t[:, j * N + cs:j * N + ce],
                        start=True,
                        stop=True,
                    )
                    o = offs[eng]
                    if eng == "v":
                        nc.vector.tensor_copy(tiles[eng][:, o:o + w],
                                              ps[:, :w])
                    else:
                        nc.scalar.copy(tiles[eng][:, o:o + w], ps[:, :w])
                    offs[eng] = o + w
                nc.sync.dma_start(outv[j], sv[:])
                nc.sync.dma_start(outa[j], sa[:])
    nc.compile()
    return nc


def _get_nc(variant):
    if variant not in _nc_cache:
        _nc_cache[variant] = build_bass(variant)
    return _nc_cache[variant]


def host_inputs(evecs, variant="tri16"):
    """Per-core input maps. Core c -> (b=c//2, half=c%2)."""
    in_maps = []
    for c in range(NCORES):
        b, half = divmod(c, 2)
        X = np.asarray(evecs[b, 0], dtype=np.float32)      # [1024, 16]
        xr16 = np.ascontiguousarray(X.T).astype(np.float16)  # [16, 1024]
        xm16 = np.zeros((KC, LEV, N), np.float16)
        for j in range(LEV):
            kmax = half * LEV + j + 1   # number of live eigvecs at level
            xm16[:kmax, j, :] = xr16[:kmax]
        in_maps.append({
            "xr": xr16,
            "xm": np.ascontiguousarray(xm16.reshape(KC, LEV * N)),
        })
    return in_maps


def unpack(results):
    """Assemble the full fp32 output from per-core packed fp16 buffers."""
    full = np.empty((B, NLEV, N, N), np.float32)
    for c in range(NCORES):
        b, half = divmod(c, 2)
        ov = results[c]["outv"]   # [LEV, 128, VW] fp16
        oa = results[c]["outa"]   # [LEV, 128, AW] fp16
        for j in range(LEV):
            slab = full[b, half * LEV + j]
            off = 0
            for i, cs, ce in VPIECES:
                w = ce - cs
                slab[i * P:(i + 1) * P, cs:ce] = ov[j, :, off:off + w]
                off += w
            off = 0
            for i, cs, ce in APIECES:
                w = ce - cs
                slab[i * P:(i + 1) * P, cs:ce] = oa[j, :, off:off + w]
                off += w
    # mirror the strictly-lower blocks from the upper triangle
    V = full.reshape(B, NLEV, RC, P, RC, P)
    for i2 in range(RC):
        for j2 in range(i2):
            V[:, :, i2, :, j2, :] = V[:, :, j2, :, i2, :].swapaxes(-2, -1)
    return full


def run(evecs, trace=False, mm_dtype="tri16", **spmd_kwargs):
    from concourse.bass_utils import run_bass_kernel_spmd

    variant = "tri16"
    nc = _get_nc(variant)
    in_maps = host_inputs(evecs, variant)
    r = run_bass_kernel_spmd(
        nc, in_maps, core_ids=list(range(NCORES)), trace=trace, **spmd_kwargs
    )
    return unpack(r.results), r


def kernel(**inputs):
    evecs = np.asarray(inputs["evecs"])
    full, _ = run(evecs)
    return full
